# revision 1
# baseline (speedup 1.0000x reference)
"""Trainium2 Bass kernel for nn_CausalSelfAttention_22016002359635.

Reference computation (B=64, T=512, D=1024, DK=16):
    q = x @ Wq + bq                       # [B,T,16]
    k = x @ Wk + bk                       # [B,T,16]
    v = x @ Wv + bv                       # [B,T,1024]
    k = where(padding_mask, -1e24, k)     # replace k rows at padded positions
    att = (q @ k^T) * 4.0                 # sqrt(16)
    att = where(causal_upper, -1e24, att)
    out = softmax(att, axis=-1) @ v

Sharding: data-parallel over batch, 8 batches per NeuronCore x 8 cores.

Device algorithm per (core, batch):
  - x^T is pre-transposed on the host and DMA'd twice: once as exact fp32
    (feeding the Q/K chain) and once as float32r (feeding the V matmul;
    fp32r streams at 4x the fp32 rate on the PE).
  - One fused fp32 projection matmul computes [4*Wq | rowsum(4*Wq) | Wk]^T
    @ x^T, yielding q^T (pre-scaled by sqrt(dk)=4, exact power of two), a
    4*qsum row, and k^T in one PSUM tile.  The padding mask is applied by
    zeroing padded columns of k^T (multiply by 0/1 mask) and adding a 17th
    contraction row (-1e24 at padded columns) against the 4*qsum row: this
    reproduces the reference's att[t, padded s] = 4*sum_d q[t,d]*(-1e24)
    semantics exactly, including its sign dependence on sum(q).  The Q/K/att
    chain stays fp32 because fp32r noise can flip the sign of near-zero
    qsum, which decides whether padded columns dominate the softmax.
  - Causal masking REPLACES (not adds) scores with exactly -1e24 to reproduce
    reference behaviour for rows whose entire prefix is padded (softmax then
    attends uniformly over future positions).  Diagonal blocks use
    copy_predicated; for t_tile 0 the full row range is materialized densely.
  - Softmax row max via reduce_max(negate), exp+rowsum fused on the scalar
    engine, normalization folded into the output scaling.
  - P^T via PE transposes; out = P^T.T @ v accumulated in PSUM (fp32r).
  - Batches with padding at position 0 need the dense tile-0 path; batches
    are sorted so whole SPMD slots can skip it (program is shared by all
    cores, so the dense/sparse structure must be uniform per slot).
"""

import os
import sys

for _p in ("/opt/trn_rl_repo", "/root/.axon_site/_ro/trn_rl_repo"):
    if os.path.isdir(_p) and _p not in sys.path:
        sys.path.insert(0, _p)

import numpy as np


def _ensure_ntff_hook():
    """Provide antenv.axon_hooks if the image lacks it, wiring the NTFF
    profiling hook to libaxon_pjrt.so so trace=True works under axon."""
    try:
        import antenv.axon_hooks  # noqa: F401
        return
    except ImportError:
        pass
    import types

    try:
        import antenv
    except ImportError:
        return
    holder = {"hook": None}
    try:
        sys.path.insert(0, "/root/.axon_site")
        from trn_agent_boot.trn_boot import _ntff_profile_via_ctypes
        so_path = "/opt/axon/libaxon_pjrt.so"
        if os.path.exists(so_path):
            holder["hook"] = _ntff_profile_via_ctypes(so_path)
    except Exception:
        pass
    mod = types.ModuleType("antenv.axon_hooks")
    mod.get_axon_ntff_profile_hook = lambda: holder["hook"]
    mod.set_axon_ntff_profile_hook = lambda h: holder.__setitem__("hook", h)
    sys.modules["antenv.axon_hooks"] = mod
    antenv.axon_hooks = mod


_ensure_ntff_hook()

import concourse.bass as bass
import concourse.tile as tile
from concourse import bacc, mybir
from concourse.bass import ds, ts
from concourse.bass_utils import run_bass_kernel_spmd
from concourse.masks import make_identity

F32 = mybir.dt.float32
F32R = mybir.dt.float32r
BF16 = mybir.dt.bfloat16
U8 = mybir.dt.uint8

B, T, D, DK = 64, 512, 1024, 16
NCORES = 8
NB = B // NCORES          # batches per core
NEG = -1e24               # the reference's -INF
NT = T // 128             # 4 t/s tiles per sequence
ND = D // 512             # 2 output column chunks
NK = D // 128             # 8 contraction chunks
QKM = 48                  # rows: 4*Wq (16) | 4*qsum (1) | pad | Wk at 32-47


def _build_program(nb=NB, use_f32r=True, dense_tiles=(True, False, False, False),
                   slot_dense=None, with_bias_qk=False, with_bias_v=False):
    """Build and compile the per-core Bass program (SPMD across 8 cores)."""
    nc = bacc.Bacc("TRN2", target_bir_lowering=False, debug=False,
                   num_devices=NCORES)

    xt8 = nc.dram_tensor("xt8", [nb, D, T], F32, kind="ExternalInput").ap()
    xth8 = nc.dram_tensor("xth8", [nb, D, T], BF16, kind="ExternalInput").ap()
    xtl8 = nc.dram_tensor("xtl8", [nb, D, T], BF16, kind="ExternalInput").ap()
    wqkh = nc.dram_tensor("wqkh", [D, QKM], BF16, kind="ExternalInput").ap()
    wqkl = nc.dram_tensor("wqkl", [D, QKM], BF16, kind="ExternalInput").ap()
    wv = nc.dram_tensor("wv", [D, D], F32, kind="ExternalInput").ap()
    pmul = nc.dram_tensor("pmul", [nb, T], F32, kind="ExternalInput").ap()
    padd = nc.dram_tensor("padd", [nb, T], F32, kind="ExternalInput").ap()
    causal = nc.dram_tensor("causal", [128, 128], U8, kind="ExternalInput").ap()
    ident128 = nc.dram_tensor("ident128", [128, 128], F32, kind="ExternalInput").ap()
    if with_bias_qk:
        bqk = nc.dram_tensor("bqk", [1, QKM], F32, kind="ExternalInput").ap()
    if with_bias_v:
        bv = nc.dram_tensor("bv", [1, D], F32, kind="ExternalInput").ap()
    out8 = nc.dram_tensor("out8", [nb, T, D], F32, kind="ExternalOutput").ap()

    MDT = F32R if use_f32r else F32
    if slot_dense is None:
        slot_dense = [True] * nb

    with tile.TileContext(nc) as tc:
        with (
            tc.tile_pool(name="consts", bufs=1) as consts,
            tc.tile_pool(name="xpool", bufs=2) as xpool,
            tc.tile_pool(name="xtpool", bufs=2) as xtpool,
            tc.tile_pool(name="vpool", bufs=2) as vpool,
            tc.tile_pool(name="qkpool", bufs=2) as qkpool,
            tc.tile_pool(name="smpool", bufs=8) as smpool,
            tc.tile_pool(name="expool", bufs=3) as expool,
            tc.tile_pool(name="extpool", bufs=3) as extpool,
            tc.tile_pool(name="opool", bufs=3) as opool,
            tc.tile_pool(name="pstr", bufs=2, space="PSUM") as pstr,
            tc.tile_pool(name="psqk", bufs=1, space="PSUM") as psqk,
            tc.tile_pool(name="psv", bufs=1, space="PSUM") as psv,
            tc.tile_pool(name="psatt", bufs=1, space="PSUM") as psatt,
            tc.tile_pool(name="psout", bufs=1, space="PSUM") as psout,
        ):
            # ---- resident constants ----
            wv_sb = consts.tile([128, NK, D], MDT)
            wv_r = wv.rearrange("(c p) d -> p c d", p=128).bitcast(MDT)
            wqkh_sb = consts.tile([128, NK, QKM], BF16)
            nc.sync.dma_start(out=wqkh_sb,
                              in_=wqkh.rearrange("(c p) m -> p c m", p=128))
            wqkl_sb = consts.tile([128, NK, QKM], BF16)
            nc.sync.dma_start(out=wqkl_sb,
                              in_=wqkl.rearrange("(c p) m -> p c m", p=128))
            causal_sb = consts.tile([128, 128], U8)
            nc.sync.dma_start(out=causal_sb, in_=causal)
            neginf_sb = consts.tile([128, 512], F32)
            nc.vector.memset(neginf_sb, NEG)
            ident = consts.tile([128, 128], F32)
            make_identity(nc, ident)
            identr = consts.tile([128, 128], MDT, name="identr")
            nc.sync.dma_start(out=identr, in_=ident128.bitcast(MDT))
            if with_bias_qk:
                ones_sb = consts.tile([1, 512], F32)
                nc.vector.memset(ones_sb, 1.0)
                bqk_sb = consts.tile([1, QKM], F32)
                nc.sync.dma_start(out=bqk_sb, in_=bqk)
            if with_bias_v:
                ones_v = consts.tile([1, 512], MDT)
                nc.vector.memset(ones_v, 1.0)
            if with_bias_v:
                bv_sb = consts.tile([1, D], MDT)
                nc.sync.dma_start(out=bv_sb, in_=bv.bitcast(MDT))

            for b in range(nb):
                # ---- x^T comes pre-transposed from the host ----
                xtb = xt8[b].rearrange("(c p) t -> p c t", p=128)
                xthb = xth8[b].rearrange("(c p) t -> p c t", p=128)
                xtlb = xtl8[b].rearrange("(c p) t -> p c t", p=128)
                xTh = xtpool.tile([128, NK, T], BF16, name="xTh")
                xTl = xtpool.tile([128, NK, T], BF16, name="xTl")
                xTr = xtpool.tile([128, NK, T], MDT, name="xTr")
                if b == 0:
                    # batch 0: land the small bf16 q/k streams first so the
                    # qk matmul chain starts immediately; the f32r x and Wv
                    # chunks (needed later, by the V matmuls) stream behind
                    for k in range(NK):
                        nc.sync.dma_start(out=xTh[:, k, :], in_=xthb[:, k, :])
                        nc.sync.dma_start(out=xTl[:, k, :], in_=xtlb[:, k, :])
                    for k in range(NK):
                        nc.sync.dma_start(out=xTr[:, k, :],
                                          in_=xtb[:, k, :].bitcast(MDT))
                        nc.sync.dma_start(out=wv_sb[:, k, :], in_=wv_r[:, k, :])
                else:
                    for k in range(NK):
                        nc.sync.dma_start(out=xTh[:, k, :], in_=xthb[:, k, :])
                        nc.sync.dma_start(out=xTl[:, k, :], in_=xtlb[:, k, :])
                        nc.sync.dma_start(out=xTr[:, k, :],
                                          in_=xtb[:, k, :].bitcast(MDT))

                # ---- fused q/k/qsum projection: qkps[m, t] ----
                # Dekker bf16 hi/lo: x@W = xh@wh + xh@wl + xl@wh (+ ~2^-17)
                qkps = psqk.tile([QKM, T], F32, name="qkps")
                for k in range(NK):
                    last = k == NK - 1 and not with_bias_qk
                    nc.tensor.matmul(qkps, wqkh_sb[:, k, :], xTh[:, k, :],
                                     start=(k == 0), stop=False)
                    nc.tensor.matmul(qkps, wqkl_sb[:, k, :], xTh[:, k, :],
                                     start=False, stop=False)
                    nc.tensor.matmul(qkps, wqkh_sb[:, k, :], xTl[:, k, :],
                                     start=False, stop=last)
                if with_bias_qk:
                    nc.tensor.matmul(qkps, bqk_sb, ones_sb,
                                     start=False, stop=True)

                kt = qkpool.tile([DK + 1, T], F32, name="kt")
                pm = qkpool.tile([DK, T], F32, name="pm")
                pmb = pmul[b:b + 1, :]
                nc.gpsimd.dma_start(
                    out=pm,
                    in_=bass.AP(tensor=pmb.tensor, offset=pmb.offset,
                                ap=[[0, DK]] + list(pmb.ap[1:])))
                nc.vector.tensor_mul(kt[0:DK, :], qkps[32:48, :], pm)
                nc.sync.dma_start(out=kt[DK:DK + 1, :], in_=padd[b:b + 1, :])

                # Dekker split of q/k into bf16 hi/lo pairs so the att matmul
                # streams at 1 cycle/row instead of fp32's 4, while q.k stays
                # exact to ~1e-5 (only the lo*lo term is dropped):
                #   q.k = qh.kh + qh.kl + ql.kh  (+ ql.kl ~ 2^-18)
                # Rows at 32-aligned bases; pad rows zeroed (memset) so they
                # contribute exact zeros to the contraction.
                qtx = qkpool.tile([81, T], BF16, name="qtx")
                ktx = qkpool.tile([81, T], BF16, name="ktx")
                nc.vector.memset(qtx, 0.0)
                nc.vector.memset(ktx, 0.0)
                nc.vector.tensor_copy(qtx[0:17, :], qkps[0:17, :])
                nc.vector.tensor_copy(qtx[32:49, :], qkps[0:17, :])
                nc.vector.tensor_sub(qtx[64:81, :], qkps[0:17, :], qtx[0:17, :])
                nc.vector.tensor_copy(ktx[0:17, :], kt)
                nc.vector.tensor_sub(ktx[32:49, :], kt, ktx[0:17, :])
                nc.vector.tensor_copy(ktx[64:81, :], kt)

                # ---- v = x @ Wv (+ bv) ----
                vsb = vpool.tile([128, NT, D], MDT)
                for i in range(NT):
                    vps = [psv.tile([128, 512], F32, name=f"vps{dj}")
                           for dj in range(ND)]
                    for k in range(NK):
                        for dj in range(ND):
                            nc.tensor.matmul(
                                vps[dj], xTr[:, k, ts(i, 128)],
                                wv_sb[:, k, ts(dj, 512)],
                                start=(k == 0),
                                stop=(k == NK - 1 and not with_bias_v))
                    for dj in range(ND):
                        if with_bias_v:
                            nc.tensor.matmul(vps[dj], ones_v[:, 0:128],
                                             bv_sb[:, ts(dj, 512)],
                                             start=False, stop=True)
                        nc.scalar.copy(vsb[:, i, ts(dj, 512)], vps[dj])

                # ---- attention row-tiles ----
                for i in range(NT):
                    nmm = (i + 1) * 128            # columns with real scores
                    dense_i = dense_tiles[i] and (i > 0 or slot_dense[b])
                    esm = T if dense_i else nmm   # softmax/PV domain
                    atps = psatt.tile([128, 512], F32, name="atps")
                    nc.tensor.matmul(atps[:, 0:nmm], qtx[:, ts(i, 128)],
                                     ktx[:, 0:nmm], start=True, stop=True)
                    # replace upper-triangular part of diagonal block with -1e24
                    nc.vector.copy_predicated(
                        atps[:, ts(i, 128)], causal_sb, neginf_sb[:, 0:128])
                    if esm > nmm:
                        # fill fully-masked future blocks with exactly -1e24
                        nc.vector.tensor_copy(
                            atps[:, nmm:esm], neginf_sb[:, 0:esm - nmm])
                    negmax = smpool.tile([128, 1], F32, name="negmax")
                    nc.vector.reduce_max(negmax, atps[:, 0:esm],
                                         axis=mybir.AxisListType.X, negate=True)
                    ex = expool.tile([128, 512], MDT, name="ex")
                    rsum = smpool.tile([128, 1], F32, name="rsum")
                    nc.scalar.activation(
                        ex[:, 0:esm], atps[:, 0:esm],
                        mybir.ActivationFunctionType.Exp,
                        bias=negmax, accum_out=rsum)
                    rrs = smpool.tile([128, 1], F32, name="rrs")
                    nc.vector.reciprocal(rrs, rsum)

                    # P^T via PE transposes (one PSUM bank per t-tile)
                    nsc = esm // 128
                    trp2 = pstr.tile([128, 512], MDT, name="trp")
                    for s in range(nsc):
                        nc.tensor.transpose(
                            trp2[:, ts(s, 128)], ex[:, ts(s, 128)], identr)
                    exT = extpool.tile([128, 512], MDT, name="exT")
                    nc.vector.tensor_copy(exT[:, 0:esm], trp2[:, 0:esm])

                    ops = [psout.tile([128, 512], F32, name=f"ops{dj}")
                           for dj in range(ND)]
                    for s in range(nsc):
                        for dj in range(ND):
                            nc.tensor.matmul(
                                ops[dj], exT[:, ts(s, 128)],
                                vsb[:, s, ts(dj, 512)],
                                start=(s == 0), stop=(s == nsc - 1))
                    for dj in range(ND):
                        osb = opool.tile([128, 512], F32, name="osb")
                        nc.scalar.activation(
                            osb, ops[dj], mybir.ActivationFunctionType.Copy,
                            bias=0.0, scale=rrs)
                        nc.sync.dma_start(
                            out=out8[b, ts(i, 128), ts(dj, 512)], in_=osb)

    nc.compile()
    return nc


def _host_prep(x, padding_mask, Wq, bq, Wk, bk, Wv, bv):
    """Precompute small host-side tensors (masks, fused qk weight)."""
    import ml_dtypes
    xt = np.ascontiguousarray(
        np.asarray(x, dtype=np.float32).transpose(0, 2, 1))
    xth = xt.astype(ml_dtypes.bfloat16)
    xtl = (xt - xth.astype(np.float32)).astype(ml_dtypes.bfloat16)
    Wv = np.ascontiguousarray(np.asarray(Wv), dtype=np.float32)
    Wq = np.asarray(Wq, dtype=np.float32)
    Wk = np.asarray(Wk, dtype=np.float32)
    bq = np.asarray(bq, dtype=np.float32)
    bk = np.asarray(bk, dtype=np.float32)
    bv = np.asarray(bv, dtype=np.float32)
    pmask = np.asarray(padding_mask).reshape(B, T).astype(bool)

    wq4 = (Wq.astype(np.float64) * 4.0).astype(np.float32)
    wqk = np.zeros((D, QKM), dtype=np.float32)
    wqk[:, 0:DK] = wq4
    wqk[:, DK] = wq4.astype(np.float64).sum(axis=1).astype(np.float32)
    wqk[:, 32:48] = Wk
    wqk = np.ascontiguousarray(wqk)
    import ml_dtypes as _mld
    wqkh = wqk.astype(_mld.bfloat16)
    wqkl = (wqk - wqkh.astype(np.float32)).astype(_mld.bfloat16)

    pmul = np.where(pmask, np.float32(0.0), np.float32(1.0))
    padd = np.where(pmask, np.float32(NEG), np.float32(0.0))

    r = np.arange(128)
    causal = (r[None, :] > r[:, None]).astype(np.uint8)
    causal = np.ascontiguousarray(causal)
    ident128 = np.eye(128, dtype=np.float32)

    bq4 = (bq.astype(np.float64) * 4.0).astype(np.float32)
    bqk = np.zeros((1, QKM), dtype=np.float32)
    bqk[0, 0:DK] = bq4
    bqk[0, DK] = bq4.astype(np.float64).sum()
    bqk[0, 32:48] = bk
    with_bias_qk = bool(np.any(bq != 0) or np.any(bk != 0))
    with_bias_v = bool(np.any(bv != 0))

    # a t-tile needs the dense (full row range) path iff some row in it can
    # have its entire prefix padded (then the reference's softmax max comes
    # from the causal -1e24 region and mass spills onto future positions).
    prefix_all = np.cumprod(pmask, axis=1).astype(bool)   # [B, T]
    dense_tiles = tuple(
        bool(prefix_all[:, it * 128: (it + 1) * 128].any()) if it > 0 else True
        for it in range(NT))
    dense_b = prefix_all[:, 0]                            # tile-0 dense per batch
    # sort dense batches first and deal slot-major so whole slots are sparse
    order = np.argsort(~dense_b, kind="stable").astype(np.int64)
    slot_dense = [bool(dense_b[order[j * NCORES:(j + 1) * NCORES]].any())
                  for j in range(B // NCORES)]

    return dict(ident128=ident128, xt=xt, xth=xth, xtl=xtl, wqkh=wqkh, wqkl=wqkl, wqk=wqk, wv=Wv, pmul=pmul, padd=padd, causal=causal,
                order=order, slot_dense=slot_dense,
                bqk=np.ascontiguousarray(bqk),
                bv=np.ascontiguousarray(bv.reshape(1, D)),
                with_bias_qk=with_bias_qk, with_bias_v=with_bias_v,
                dense_tiles=dense_tiles)


def _in_maps(prep, nb=NB, ncores=NCORES):
    maps = []
    for c in range(ncores):
        idx = prep["order"][[j * ncores + c for j in range(nb)]]
        m = {
            "xt8": np.ascontiguousarray(prep["xt"][idx]),
            "xth8": np.ascontiguousarray(prep["xth"][idx]),
            "xtl8": np.ascontiguousarray(prep["xtl"][idx]),
            "wqkh": prep["wqkh"],
            "wqkl": prep["wqkl"],
            "wv": prep["wv"],
            "pmul": np.ascontiguousarray(prep["pmul"][idx]),
            "padd": np.ascontiguousarray(prep["padd"][idx]),
            "causal": prep["causal"],
            "ident128": prep["ident128"],
        }
        if prep["with_bias_qk"]:
            m["bqk"] = prep["bqk"]
        if prep["with_bias_v"]:
            m["bv"] = prep["bv"]
        maps.append(m)
    return maps


def run(inputs, use_f32r=True, trace=False, tmpdir=None):
    """Build + run on 8 NeuronCores; returns (full_output, BassKernelResults)."""
    prep = _host_prep(**inputs)
    nc = _build_program(nb=NB, use_f32r=use_f32r,
                        dense_tiles=prep["dense_tiles"],
                        slot_dense=prep["slot_dense"],
                        with_bias_qk=prep["with_bias_qk"],
                        with_bias_v=prep["with_bias_v"])
    maps = _in_maps(prep)
    try:
        res = run_bass_kernel_spmd(nc, maps, list(range(NCORES)),
                                   trace=trace, tmpdir=tmpdir)
    except Exception:
        # transient device errors (e.g. a wedged core from a prior run)
        # usually clear on retry
        res = run_bass_kernel_spmd(nc, maps, list(range(NCORES)),
                                   trace=trace, tmpdir=tmpdir)
    out = np.empty((B, T, D), dtype=np.float32)
    for c in range(NCORES):
        idx = prep["order"][[j * NCORES + c for j in range(NB)]]
        out[idx] = res.results[c]["out8"]
    return out, res


def kernel(**inputs):
    out, _ = run(inputs, use_f32r=True)
    return out



# revision 9
# speedup vs baseline: 1.0903x; 1.0903x over previous
"""Trainium2 Bass kernel for nn_CausalSelfAttention_22016002359635.

Reference computation (B=64, T=512, D=1024, DK=16):
    q = x @ Wq + bq                       # [B,T,16]
    k = x @ Wk + bk                       # [B,T,16]
    v = x @ Wv + bv                       # [B,T,1024]
    k = where(padding_mask, -1e24, k)     # replace k rows at padded positions
    att = (q @ k^T) * 4.0                 # sqrt(16)
    att = where(causal_upper, -1e24, att)
    out = softmax(att, axis=-1) @ v

Sharding: data-parallel over batch, 8 batches per NeuronCore x 8 cores.

Device algorithm per (core, batch):
  - x^T is pre-transposed on the host and DMA'd as bf16 hi/lo Dekker pair
    (feeding the Q/K projection) plus fp32 bits typed float32r (feeding the
    V matmul; fp32r streams at 4x the fp32 rate on the PE).
  - Fused projection W' = [4*Wq | rowsum(4*Wq) | pad | Wk] (48 rows) is
    applied with a 2-pass Dekker scheme: pass 1 packs [W'h | W'l] as a
    96-wide stationary over the xh stream; pass 2 accumulates W'h over the
    xl stream into rows 0:48.  A vector add of the two 48-row PSUM halves
    yields q (pre-scaled by sqrt(dk)=4), an exact 4*qsum row, and k, all at
    near-fp32 precision (only the xl@W'l term is dropped).  Exactness of
    4*qsum matters: the reference's att[t, padded s] = 4*sum_d q[t,d]*(-1e24)
    makes every row's behaviour flip on sign(qsum), and min |4*qsum| over
    this dataset is ~1.1e-4.
  - The attention matmul itself runs as a single fp32r pass (17 contraction
    rows: 16 q rows + the 4*qsum row against k rows + a -1e24 padding row).
    fp32r's ~2^-11 relative rounding cannot flip sign(qsum) (the error is
    relative to the already-exact value) and score noise ~7e-3 is far under
    the softmax tolerance.  Streams are kept >= 256 columns to avoid the
    fp32r short-stream penalty; the overshot columns are overwritten by the
    -1e24 causal/future fill that is needed anyway.
  - Causal masking REPLACES (not adds) scores with exactly -1e24 to
    reproduce reference behaviour for rows whose entire prefix is padded
    (softmax then attends uniformly over future positions).  Batches with
    padding at position 0 need the dense tile-0 path; batches are sorted so
    whole SPMD slots can skip it (the program is shared by all cores, so the
    dense/sparse structure must be uniform per slot).
  - Softmax row max via reduce_max(negate), exp+rowsum fused on the scalar
    engine (bf16 weights), P^T via bf16 PE transposes (1.0 cyc/row vs
    fp32r's 1.5), out = P^T.T @ v in bf16 with the normalization folded
    into the output scaling.  v is stored bf16 (0.2% rel, well under the
    2e-2 budget).
  - Program order interleaves batch b's attention tiles with batch b+1's
    V-projection chunks and QK passes: the per-engine queues execute in
    order, so without filler the PE would stall ~1.3us per tile waiting on
    the vector/scalar softmax chain, dropping it out of its top p-state.
"""

import os
import sys

for _p in ("/opt/trn_rl_repo", "/root/.axon_site/_ro/trn_rl_repo"):
    if os.path.isdir(_p) and _p not in sys.path:
        sys.path.insert(0, _p)

import numpy as np


def _ensure_ntff_hook():
    """Provide antenv.axon_hooks if the image lacks it, wiring the NTFF
    profiling hook to libaxon_pjrt.so so trace=True works under axon."""
    try:
        import antenv.axon_hooks  # noqa: F401
        return
    except ImportError:
        pass
    import types

    try:
        import antenv
    except ImportError:
        return
    holder = {"hook": None}
    try:
        sys.path.insert(0, "/root/.axon_site")
        from trn_agent_boot.trn_boot import _ntff_profile_via_ctypes
        so_path = "/opt/axon/libaxon_pjrt.so"
        if os.path.exists(so_path):
            holder["hook"] = _ntff_profile_via_ctypes(so_path)
    except Exception:
        pass
    mod = types.ModuleType("antenv.axon_hooks")
    mod.get_axon_ntff_profile_hook = lambda: holder["hook"]
    mod.set_axon_ntff_profile_hook = lambda h: holder.__setitem__("hook", h)
    sys.modules["antenv.axon_hooks"] = mod
    antenv.axon_hooks = mod


_ensure_ntff_hook()

import concourse.bass as bass
import concourse.tile as tile
from concourse import bacc, mybir
from concourse.bass import ds, ts
from concourse.bass_utils import run_bass_kernel_spmd

F32 = mybir.dt.float32
F32R = mybir.dt.float32r
BF16 = mybir.dt.bfloat16
U8 = mybir.dt.uint8

B, T, D, DK = 64, 512, 1024, 16
NCORES = 8
NB = B // NCORES          # batches per core
NEG = -1e24               # the reference's -INF
NT = T // 128             # 4 t/s tiles per sequence
ND = D // 512             # 2 output column chunks
NK = D // 128             # 8 contraction chunks
QKM = 48                  # W' rows: 4*Wq (16) | 4*qsum (1) | pad | Wk at 32-47
QKW = 128                 # packed [W'h | pad | W'l | pad] stationary width


def _build_program(nb=NB, use_f32r=True, dense_tiles=(True, False, False, False),
                   slot_dense=None, with_bias_qk=False, with_bias_v=False):
    """Build and compile the per-core Bass program (SPMD across 8 cores)."""
    nc = bacc.Bacc("TRN2", target_bir_lowering=False, debug=False,
                   num_devices=NCORES)

    xt8 = nc.dram_tensor("xt8", [nb, D, T], F32, kind="ExternalInput").ap()
    xth8 = nc.dram_tensor("xth8", [nb, D, T], BF16, kind="ExternalInput").ap()
    xtl8 = nc.dram_tensor("xtl8", [nb, D, T], BF16, kind="ExternalInput").ap()
    wqkhl = nc.dram_tensor("wqkhl", [D, QKW], BF16, kind="ExternalInput").ap()
    wv = nc.dram_tensor("wv", [D, D], F32, kind="ExternalInput").ap()
    pmul = nc.dram_tensor("pmul", [nb, T], F32, kind="ExternalInput").ap()
    padd = nc.dram_tensor("padd", [nb, T], F32, kind="ExternalInput").ap()
    causal = nc.dram_tensor("causal", [128, 128], U8, kind="ExternalInput").ap()
    identb = nc.dram_tensor("identb", [128, 128], BF16, kind="ExternalInput").ap()
    if with_bias_qk:
        bqk = nc.dram_tensor("bqk", [1, QKM], F32, kind="ExternalInput").ap()
    if with_bias_v:
        bv = nc.dram_tensor("bv", [1, D], F32, kind="ExternalInput").ap()
    out8 = nc.dram_tensor("out8", [nb, T, D], F32, kind="ExternalOutput").ap()

    MDT = F32R if use_f32r else F32
    if slot_dense is None:
        slot_dense = [True] * nb

    wv_r = wv.rearrange("(c p) d -> p c d", p=128).bitcast(MDT)

    with tile.TileContext(nc) as tc:
        with (
            tc.tile_pool(name="consts", bufs=1) as consts,
            tc.tile_pool(name="xtpool", bufs=2) as xtpool,
            tc.tile_pool(name="vpool", bufs=2) as vpool,
            tc.tile_pool(name="qkpool", bufs=2) as qkpool,
            tc.tile_pool(name="smpool", bufs=8) as smpool,
            tc.tile_pool(name="expool", bufs=3) as expool,
            tc.tile_pool(name="extpool", bufs=3) as extpool,
            tc.tile_pool(name="opool", bufs=3) as opool,
            tc.tile_pool(name="psqk", bufs=1, space="PSUM") as psqk,
            tc.tile_pool(name="psatt", bufs=2, space="PSUM") as psatt,
            tc.tile_pool(name="psv", bufs=1, space="PSUM") as psv,
            tc.tile_pool(name="pstr", bufs=1, space="PSUM") as pstr,
            tc.tile_pool(name="psout", bufs=1, space="PSUM") as psout,
        ):
            # ---- resident constants (issued before any x traffic) ----
            wqk_sb = consts.tile([128, NK, QKW], BF16)
            nc.sync.dma_start(out=wqk_sb,
                              in_=wqkhl.rearrange("(c p) m -> p c m", p=128))
            causal_sb = consts.tile([128, 128], U8)
            nc.sync.dma_start(out=causal_sb, in_=causal)
            identb_sb = consts.tile([128, 128], BF16)
            nc.sync.dma_start(out=identb_sb, in_=identb)
            neginf_sb = consts.tile([128, 512], F32)
            nc.vector.memset(neginf_sb, NEG)
            wv_sb = consts.tile([128, NK, D], MDT)
            if with_bias_qk:
                ones_sb = consts.tile([1, 512], F32)
                nc.vector.memset(ones_sb, 1.0)
                bqk_sb = consts.tile([1, QKM], F32)
                nc.sync.dma_start(out=bqk_sb, in_=bqk)
            if with_bias_v:
                ones_v = consts.tile([1, 128], MDT)
                nc.vector.memset(ones_v, 1.0)
                bv_sb = consts.tile([1, D], MDT)
                nc.sync.dma_start(out=bv_sb, in_=bv.bitcast(MDT))

            # per-batch state kept across the interleaved emission
            st = {}

            def emit_dma_x(b, with_wv=False):
                xtb = xt8[b].rearrange("(c p) t -> p c t", p=128)
                xthb = xth8[b].rearrange("(c p) t -> p c t", p=128)
                xtlb = xtl8[b].rearrange("(c p) t -> p c t", p=128)
                xTh = xtpool.tile([128, NK, T], BF16, name="xTh")
                xTl = xtpool.tile([128, NK, T], BF16, name="xTl")
                xTr = xtpool.tile([128, NK, T], MDT, name="xTr")
                # bf16 q/k streams first so the qk chain starts immediately;
                # the f32r x (and wv on batch 0) stream behind, paired per
                # chunk in the order the V matmul consumes them.
                for k in range(NK):
                    nc.sync.dma_start(out=xTh[:, k, :], in_=xthb[:, k, :])
                    nc.sync.dma_start(out=xTl[:, k, :], in_=xtlb[:, k, :])
                for k in range(NK):
                    nc.sync.dma_start(out=xTr[:, k, :],
                                      in_=xtb[:, k, :].bitcast(MDT))
                    if with_wv:
                        nc.sync.dma_start(out=wv_sb[:, k, :], in_=wv_r[:, k, :])
                st[b] = {"xTh": xTh, "xTl": xTl, "xTr": xTr}

            def emit_qk(b):
                s = st[b]
                # 2-pass Dekker: [W'h|W'l]@xh then +W'h@xl into rows 0:48
                qkps = psqk.tile([QKW, T], F32, name="qkps")
                for k in range(NK):
                    nc.tensor.matmul(qkps, wqk_sb[:, k, :], s["xTh"][:, k, :],
                                     start=(k == 0), stop=(k == NK - 1))
                for k in range(NK):
                    last = k == NK - 1 and not with_bias_qk
                    nc.tensor.matmul(qkps[0:QKM, :], wqk_sb[:, k, 0:QKM],
                                     s["xTl"][:, k, :], start=False, stop=last,
                                     skip_group_check=True)
                if with_bias_qk:
                    nc.tensor.matmul(qkps[0:QKM, :], bqk_sb, ones_sb,
                                     start=False, stop=True,
                                     skip_group_check=True)
                # merge halves: DVE reads at most one PSUM input and SBUF
                # TensorTensor operands must share a start partition, so
                # stage the lo half through base-0 SBUF copies first.
                # qm rows 0:16 = 4q, row 16 = 4qsum; kt rows 0:16 = k*pmask
                loq = qkpool.tile([DK + 1, T], F32, name="loq")
                nc.vector.tensor_copy(loq, qkps[64:64 + DK + 1, :])
                lok = qkpool.tile([DK, T], F32, name="lok")
                nc.vector.tensor_copy(lok, qkps[96:112, :])
                qm = qkpool.tile([DK + 1, T], MDT, name="qm")
                nc.vector.tensor_add(qm, qkps[0:DK + 1, :], loq)
                kt = qkpool.tile([DK + 1, T], MDT, name="kt")
                pm = qkpool.tile([DK, T], F32, name="pm")
                pmb = pmul[b:b + 1, :]
                nc.gpsimd.dma_start(
                    out=pm,
                    in_=bass.AP(tensor=pmb.tensor, offset=pmb.offset,
                                ap=[[0, DK]] + list(pmb.ap[1:])))
                nc.vector.tensor_add(kt[0:DK, :], qkps[32:48, :], lok)
                nc.vector.tensor_mul(kt[0:DK, :], kt[0:DK, :], pm)
                nc.sync.dma_start(out=kt[DK:DK + 1, :],
                                  in_=padd[b:b + 1, :].bitcast(MDT))
                s["qm"] = qm
                s["kt"] = kt
                if b == 0:
                    s["vsb"] = vpool.tile([128, NT, D], BF16, name="vsb")

            def emit_v_chunk(b, i):
                s = st[b]
                if i == 0 and "vsb" not in s:
                    s["vsb"] = vpool.tile([128, NT, D], BF16, name="vsb")
                vps = [psv.tile([128, 512], F32, name=f"vps{dj}")
                       for dj in range(ND)]
                for k in range(NK):
                    for dj in range(ND):
                        nc.tensor.matmul(
                            vps[dj], s["xTr"][:, k, ts(i, 128)],
                            wv_sb[:, k, ts(dj, 512)],
                            start=(k == 0),
                            stop=(k == NK - 1 and not with_bias_v))
                for dj in range(ND):
                    if with_bias_v:
                        nc.tensor.matmul(vps[dj], ones_v, bv_sb[:, ts(dj, 512)],
                                         start=False, stop=True)
                    nc.vector.tensor_copy(s["vsb"][:, i, ts(dj, 512)], vps[dj])

            def emit_att(b, i):
                s = st[b]
                nmm = (i + 1) * 128            # columns with real scores
                dense_i = dense_tiles[i] and (i > 0 or slot_dense[b])
                sm = T if dense_i else max(nmm, 256)   # softmax/stream domain
                atps = psatt.tile([128, 512], F32, name="atps")
                nc.tensor.matmul(atps[:, 0:sm],
                                 s["qm"][:, ts(i, 128)],
                                 s["kt"][:, 0:sm],
                                 start=True, stop=True)
                # replace upper-triangular part of diagonal block with -1e24
                nc.vector.copy_predicated(
                    atps[:, ts(i, 128)], causal_sb, neginf_sb[:, 0:128])
                if sm > nmm:
                    # fill fully-masked future blocks with exactly -1e24
                    nc.vector.tensor_copy(
                        atps[:, nmm:sm], neginf_sb[:, 0:sm - nmm])
                negmax = smpool.tile([128, 1], F32, name="negmax")
                nc.vector.reduce_max(negmax, atps[:, 0:sm],
                                     axis=mybir.AxisListType.X, negate=True)
                ex = expool.tile([128, 512], BF16, name="ex")
                rsum = smpool.tile([128, 1], F32, name="rsum")
                nc.scalar.activation(
                    ex[:, 0:sm], atps[:, 0:sm],
                    mybir.ActivationFunctionType.Exp,
                    bias=negmax, accum_out=rsum)
                rrs = smpool.tile([128, 1], F32, name="rrs")
                nc.vector.reciprocal(rrs, rsum)
                s[f"ex{i}"] = ex
                s[f"rrs{i}"] = rrs
                s[f"nsc{i}"] = sm // 128 if dense_i else nmm // 128

            def emit_tr(b, i):
                s = st[b]
                nsc = s[f"nsc{i}"]
                ex = s[f"ex{i}"]
                trp = pstr.tile([128, 512], BF16, name="trp")
                for sc in range(nsc):
                    nc.tensor.transpose(
                        trp[:, ts(sc, 128)], ex[:, ts(sc, 128)], identb_sb)
                exT = extpool.tile([128, 512], BF16, name="exT")
                nc.vector.tensor_copy(exT[:, 0:nsc * 128], trp[:, 0:nsc * 128])
                s[f"exT{i}"] = exT

            def emit_pv(b, i):
                s = st[b]
                nsc = s[f"nsc{i}"]
                exT = s[f"exT{i}"]
                ops = [psout.tile([128, 512], F32, name=f"ops{dj}")
                       for dj in range(ND)]
                for sc in range(nsc):
                    for dj in range(ND):
                        nc.tensor.matmul(
                            ops[dj], exT[:, ts(sc, 128)],
                            s["vsb"][:, sc, ts(dj, 512)],
                            start=(sc == 0), stop=(sc == nsc - 1))
                for dj in range(ND):
                    osb = opool.tile([128, 512], F32, name="osb")
                    nc.scalar.activation(
                        osb, ops[dj], mybir.ActivationFunctionType.Copy,
                        bias=0.0, scale=s[f"rrs{i}"])
                    nc.sync.dma_start(
                        out=out8[b, ts(i, 128), ts(dj, 512)], in_=osb)

            # ---- prologue: batch 0 runs un-overlapped ----
            emit_dma_x(0, with_wv=True)
            emit_qk(0)
            for i in range(NT):
                emit_v_chunk(0, i)
            if nb > 1:
                emit_dma_x(1)

            # ---- steady state: attention(b) interleaved with V/QK(b+1) ----
            for b in range(nb):
                nxt = b + 1 if b + 1 < nb else None
                emit_att(b, 0)
                emit_att(b, 1)
                if nxt is not None:
                    emit_v_chunk(nxt, 0)
                emit_tr(b, 0)
                if nxt is not None:
                    emit_v_chunk(nxt, 1)
                emit_pv(b, 0)
                emit_att(b, 2)
                emit_tr(b, 1)
                if nxt is not None:
                    emit_v_chunk(nxt, 2)
                emit_pv(b, 1)
                emit_att(b, 3)
                emit_tr(b, 2)
                if nxt is not None:
                    emit_v_chunk(nxt, 3)
                    if nxt + 1 < nb:
                        emit_dma_x(nxt + 1)
                emit_pv(b, 2)
                emit_tr(b, 3)
                if nxt is not None:
                    emit_qk(nxt)
                emit_pv(b, 3)
                del st[b]

    nc.compile()
    return nc


def _host_prep(x, padding_mask, Wq, bq, Wk, bk, Wv, bv):
    """Precompute small host-side tensors (masks, fused qk weight)."""
    import ml_dtypes
    xt = np.ascontiguousarray(
        np.asarray(x, dtype=np.float32).transpose(0, 2, 1))
    xth = xt.astype(ml_dtypes.bfloat16)
    xtl = (xt - xth.astype(np.float32)).astype(ml_dtypes.bfloat16)
    Wv = np.ascontiguousarray(np.asarray(Wv), dtype=np.float32)
    Wq = np.asarray(Wq, dtype=np.float32)
    Wk = np.asarray(Wk, dtype=np.float32)
    bq = np.asarray(bq, dtype=np.float32)
    bk = np.asarray(bk, dtype=np.float32)
    bv = np.asarray(bv, dtype=np.float32)
    pmask = np.asarray(padding_mask).reshape(B, T).astype(bool)

    wq4 = (Wq.astype(np.float64) * 4.0).astype(np.float32)
    wqk = np.zeros((D, QKM), dtype=np.float32)
    wqk[:, 0:DK] = wq4
    wqk[:, DK] = wq4.astype(np.float64).sum(axis=1).astype(np.float32)
    wqk[:, 32:48] = Wk
    wqkh = wqk.astype(ml_dtypes.bfloat16)
    wqkl = (wqk - wqkh.astype(np.float32)).astype(ml_dtypes.bfloat16)
    wqkhl = np.zeros((D, 128), dtype=ml_dtypes.bfloat16)  # [W'h|pad|W'l|pad]
    wqkhl[:, 0:QKM] = wqkh
    wqkhl[:, 64:64 + QKM] = wqkl
    wqkhl = np.ascontiguousarray(wqkhl)

    pmul = np.where(pmask, np.float32(0.0), np.float32(1.0))
    padd = np.where(pmask, np.float32(NEG), np.float32(0.0))

    r = np.arange(128)
    causal = np.ascontiguousarray(
        (r[None, :] > r[:, None]).astype(np.uint8))
    identb = np.eye(128, dtype=np.float32).astype(ml_dtypes.bfloat16)

    bq4 = (bq.astype(np.float64) * 4.0).astype(np.float32)
    bqk = np.zeros((1, QKM), dtype=np.float32)
    bqk[0, 0:DK] = bq4
    bqk[0, DK] = bq4.astype(np.float64).sum()
    bqk[0, 32:48] = bk
    with_bias_qk = bool(np.any(bq != 0) or np.any(bk != 0))
    with_bias_v = bool(np.any(bv != 0))

    # a t-tile needs the dense (full row range) path iff some row in it can
    # have its entire prefix padded (then the reference's softmax max comes
    # from the causal -1e24 region and mass spills onto future positions).
    prefix_all = np.cumprod(pmask, axis=1).astype(bool)   # [B, T]
    dense_tiles = tuple(
        bool(prefix_all[:, it * 128: (it + 1) * 128].any()) if it > 0 else True
        for it in range(NT))
    dense_b = prefix_all[:, 0]                            # tile-0 dense per batch
    # sort dense batches first and deal slot-major so whole slots are sparse
    order = np.argsort(~dense_b, kind="stable").astype(np.int64)
    slot_dense = [bool(dense_b[order[j * NCORES:(j + 1) * NCORES]].any())
                  for j in range(B // NCORES)]

    return dict(xt=xt, xth=xth, xtl=xtl, wqkhl=wqkhl, wv=Wv, pmul=pmul,
                padd=padd, causal=causal, identb=identb,
                order=order, slot_dense=slot_dense,
                bqk=np.ascontiguousarray(bqk),
                bv=np.ascontiguousarray(bv.reshape(1, D)),
                with_bias_qk=with_bias_qk, with_bias_v=with_bias_v,
                dense_tiles=dense_tiles)


def _in_maps(prep, nb=NB, ncores=NCORES):
    maps = []
    for c in range(ncores):
        idx = prep["order"][[j * ncores + c for j in range(nb)]]
        m = {
            "xt8": np.ascontiguousarray(prep["xt"][idx]),
            "xth8": np.ascontiguousarray(prep["xth"][idx]),
            "xtl8": np.ascontiguousarray(prep["xtl"][idx]),
            "wqkhl": prep["wqkhl"],
            "wv": prep["wv"],
            "pmul": np.ascontiguousarray(prep["pmul"][idx]),
            "padd": np.ascontiguousarray(prep["padd"][idx]),
            "causal": prep["causal"],
            "identb": prep["identb"],
        }
        if prep["with_bias_qk"]:
            m["bqk"] = prep["bqk"]
        if prep["with_bias_v"]:
            m["bv"] = prep["bv"]
        maps.append(m)
    return maps


def run(inputs, use_f32r=True, trace=False, tmpdir=None):
    """Build + run on 8 NeuronCores; returns (full_output, BassKernelResults)."""
    prep = _host_prep(**inputs)
    nc = _build_program(nb=NB, use_f32r=use_f32r,
                        dense_tiles=prep["dense_tiles"],
                        slot_dense=prep["slot_dense"],
                        with_bias_qk=prep["with_bias_qk"],
                        with_bias_v=prep["with_bias_v"])
    maps = _in_maps(prep)
    try:
        res = run_bass_kernel_spmd(nc, maps, list(range(NCORES)),
                                   trace=trace, tmpdir=tmpdir)
    except Exception:
        # transient device errors (e.g. a wedged core from a prior run)
        # usually clear on retry
        res = run_bass_kernel_spmd(nc, maps, list(range(NCORES)),
                                   trace=trace, tmpdir=tmpdir)
    out = np.empty((B, T, D), dtype=np.float32)
    for c in range(NCORES):
        idx = prep["order"][[j * NCORES + c for j in range(NB)]]
        out[idx] = res.results[c]["out8"]
    return out, res


def kernel(**inputs):
    out, _ = run(inputs, use_f32r=True)
    return out
